# revision 56
# baseline (speedup 1.0000x reference)
"""Trainium2 Bass kernel for AffinityDynamics attention (bf16 redesign).

reference:
    q = h @ Wq.T ; k = h @ Wk.T ; v = h @ Wv.T          (per batch)
    S = q @ k.T + tau @ tau.T                            [B, N, N]
    attn = softmax(S / sqrt(D))
    out = attn @ v                                       [B, N, D]

Shapes: B=4, N=4096, D=512, R=64, fp32 in/out.

Sharding (host-side): 8 cores = batch(4) x query-half(2). Core c handles
batch b=c//2, query rows [s*2048, (s+1)*2048) with s=c%2.

Algebraic restructure (saves both K and V projection passes):
    S   = h (Wq^T Wk) h^T = q' h^T,  q' = h G,  G = Wq^T Wk
    out = (attn h) Wv^T = z Wv^T,    z = attn h
All matmul operands are bf16 (1 cycle/row on PE, measured ~219ns per
512-col matmul vs 237ns fp32r; rel err ~6e-3 vs the 2e-2 gate), with
f32 PSUM accumulation everywhere, so only operand storage is rounded.

Key-order permutation: each core's inputs (hT columns, hrow rows, tau2
columns) are host-permuted so the core's own query half occupies key
slots [0, NQ). Attention is key-permutation invariant, and this lets
phase A (q' = h G) read its moving operand straight out of the resident
hT tiles -- no separate hTq input, and the first key blocks a core needs
are exactly the columns its phase A already waited for.

Affinity: tau is host-packed as tau2 = stack([tau^T, tau^T]) / sqrt(2)
[2R=128, N], so a single full K=128 matmul per key chunk computes
sum_r tau_m tau_n exactly (the 1/2 from doubling is folded into the
1/sqrt(2) scaling). No reliance on PE row-group pairing.

Loop structure: query-block (nb) outer, key-block (mb) inner. z^T for
the current nb accumulates directly in 4 PSUM banks across all 32 key
chunks (no DVE adds, no SBUF zacc). exp row-sums accumulate on
DVE/GPSIMD (split by key-chunk parity) from the bf16 ex tiles that also
feed the z matmuls, so softmax numerator/denominator quantization
cancels. The per-nb softmax tail (sum -> 1/sum -> broadcast -> zr scale
-> out^T = Wv^T z^T) is software-pipelined: its stages are emitted
interleaved into the next nb's first key blocks so the PE never waits
on the DVE chain except at the very last nb.
"""

import numpy as np

B, N, D, R = 4, 4096, 512, 64
NCORES = 8
NQ = N // 2          # queries per core
NBLK = 512           # query-block size
KD = D // 128        # feature chunks (4)
N_NB = NQ // NBLK    # 4 query blocks
N_MB = 8             # key blocks of 512
N_MC = 4             # key chunks per block
SCALE = 1.0 / float(np.sqrt(np.float32(D)))

_CACHE: dict = {}


def _build(reps: int = 1, sum_eng: str = "wide", qk_bufs: int = 3,
           hT_bufs: int = 1, do_sums: bool = True, do_z: bool = True,
           do_tail: bool = True, do_exp: bool = True, do_qk: bool = True,
           hr_q: str = "sync", z_mode: str = "psum", ex_bufs: int = 2,
           tail_eng: str = "vector", rb_mode: str = "gpsimd"):
    key = ("nc2", reps, sum_eng, qk_bufs, hT_bufs, do_sums, do_z, do_tail,
           do_exp, do_qk, hr_q, z_mode, ex_bufs, tail_eng, rb_mode)
    if key in _CACHE:
        return _CACHE[key]

    import concourse.bass as bass
    import concourse.tile as tile
    from concourse import bacc, mybir

    f32 = mybir.dt.float32
    bf16 = mybir.dt.bfloat16
    EXP = mybir.ActivationFunctionType.Exp

    nc = bacc.Bacc("TRN2", target_bir_lowering=False, debug=False,
                   num_devices=NCORES)

    hT_d = nc.dram_tensor("hT", [D, N], bf16, kind="ExternalInput").ap()
    hrow_d = nc.dram_tensor("hrow", [N, D], bf16, kind="ExternalInput").ap()
    g_d = nc.dram_tensor("g", [D, D], bf16, kind="ExternalInput").ap()
    wvT_d = nc.dram_tensor("wvT", [D, D], bf16, kind="ExternalInput").ap()
    tau2_d = nc.dram_tensor("tau2", [2 * R, N], bf16,
                            kind="ExternalInput").ap()
    outT_d = nc.dram_tensor("outT", [D, NQ], f32, kind="ExternalOutput").ap()

    with tile.TileContext(nc) as tc:
        with tc.tile_pool(name="res", bufs=1) as res, \
             tc.tile_pool(name="hTp", bufs=hT_bufs) as hTp, \
             tc.tile_pool(name="hrp", bufs=1) as hrp, \
             tc.tile_pool(name="wsp", bufs=1) as wsp, \
             tc.tile_pool(name="qTp", bufs=2) as qTp, \
             tc.tile_pool(name="exp", bufs=2) as expp, \
             tc.tile_pool(name="ps_z", bufs=4, space="PSUM") as ps_z, \
             tc.tile_pool(name="ps_qk", bufs=qk_bufs, space="PSUM") as ps_qk, \
             tc.tile_pool(name="ps_sum", bufs=1, space="PSUM") as ps_sum:

            def body():
                # ---- prologue DMAs, first-needed first ----------------
                # scalar queue: g (gates phase A), wv; gpsimd: tau2
                g = [wsp.tile([128, D], bf16, tag=f"g{i}", name=f"g{i}")
                     for i in range(KD)]
                for i in range(KD):
                    sl = slice(i * 128, (i + 1) * 128)
                    nc.scalar.dma_start(g[i][:], g_d[sl, :])
                tau2 = wsp.tile([128, N], bf16, tag="tau2", name="tau2")
                nc.gpsimd.dma_start(tau2[:, 0:512], tau2_d[:, 0:512])
                nc.gpsimd.dma_start(tau2[:, 512:NQ], tau2_d[:, 512:NQ])
                wv = [wsp.tile([128, D], bf16, tag=f"wv{i}", name=f"wv{i}")
                      for i in range(KD)]
                for i in range(KD):
                    sl = slice(i * 128, (i + 1) * 128)
                    nc.scalar.dma_start(wv[i][:], wvT_d[sl, :])
                nc.scalar.dma_start(tau2[:, NQ:N], tau2_d[:, NQ:N])

                # sync queue: hT in key-block column chunks (the mb loop
                # marches through columns in order; phase A nb0 needs only
                # the first chunk), hrow interleaved in row chunks
                hT = [hTp.tile([128, N], bf16, tag=f"ht{kd}", name=f"ht{kd}")
                      for kd in range(KD)]
                hr = [hrp.tile([128, D], bf16, tag=f"hr{kc}", name=f"hr{kc}")
                      for kc in range(N // 128)]
                if hr_q == "sync":
                    hr_engs = [nc.sync]
                elif hr_q == "scalar":
                    hr_engs = [nc.scalar]
                else:
                    hr_engs = [nc.scalar, nc.gpsimd]
                for mb in range(N_MB):
                    csl = slice(mb * 512, (mb + 1) * 512)
                    for kd in range(KD):
                        sl = slice(kd * 128, (kd + 1) * 128)
                        nc.sync.dma_start(hT[kd][:, csl], hT_d[sl, csl])
                    for mc in range(N_MC):
                        kc = mb * N_MC + mc
                        hr_engs[kc % len(hr_engs)].dma_start(
                            hr[kc][:], hrow_d[kc * 128:(kc + 1) * 128, :])

                # ---- constants ---------------------------------------
                ones_f = res.tile([128, 2], f32, tag="ones_f", name="ones_f")
                nc.vector.memset(ones_f[:, 0:1], 1.0)
                nc.vector.memset(ones_f[:, 1:2], 0.0)
                ones = res.tile([128, 2], bf16, tag="ones", name="ones")
                nc.vector.tensor_copy(ones[:], ones_f[:])
                onecol_f = res.tile([1, 128], f32, tag="onecol_f",
                                    name="onecol_f")
                nc.vector.memset(onecol_f[:], 1.0)
                onecol = res.tile([1, 128], bf16, tag="onecol", name="onecol")
                nc.vector.tensor_copy(onecol[:], onecol_f[:])

                # ---- phase A: q'^T for one query block ---------------
                def phase_A(nb):
                    nsl = slice(nb * NBLK, (nb + 1) * NBLK)
                    qT = [qTp.tile([128, NBLK], bf16, tag=f"qT{e}",
                                   name=f"qT{e}") for e in range(KD)]
                    for e in range(KD):
                        ps = ps_qk.tile([128, NBLK], f32, tag="qk",
                                        name="qk")
                        for kd in range(KD):
                            nc.tensor.matmul(
                                ps[:], g[kd][:, e * 128:(e + 1) * 128],
                                hT[kd][:, nsl],
                                start=(kd == 0), stop=(kd == KD - 1))
                        nc.vector.tensor_copy(qT[e][:], ps[:])
                    return qT

                # ---- per-nb softmax/output tail, staged --------------
                def make_tail(nb, zps, exacc, exacc2, srp, exaccw):
                    nsl = slice(nb * NBLK, (nb + 1) * NBLK)
                    st = {}

                    def t_exr():
                        if sum_eng in ("pe", "wide"):
                            return
                        st["exr"] = expp.tile([128, NBLK], bf16, tag="exr",
                                              name="exr")
                        if exacc2 is not None:
                            nc.vector.tensor_add(st["exr"][:], exacc[:],
                                                 exacc2[:])
                        else:
                            nc.vector.tensor_copy(st["exr"][:], exacc[:])

                    def t_sum():
                        if sum_eng == "pe":
                            srp_ = srp
                        elif sum_eng == "wide":
                            srp_ = ps_sum.tile([2, NBLK], f32, tag="sm",
                                               name="sm")
                            for a in range(2):
                                for c in range(N_MC):
                                    nc.tensor.matmul(
                                        srp_[:], ones[:],
                                        exaccw[a][:,
                                                  c * NBLK:(c + 1) * NBLK],
                                        start=(a == 0 and c == 0),
                                        stop=(a == 1 and c == N_MC - 1))
                        else:
                            srp_ = ps_sum.tile([2, NBLK], f32, tag="sm",
                                               name="sm")
                            nc.tensor.matmul(srp_[:], ones[:], st["exr"][:],
                                             start=True, stop=True)
                        recf = expp.tile([1, NBLK], f32, tag="recf",
                                         name="recf")
                        nc.vector.reciprocal(recf[:], srp_[0:1, :])
                        st["recf"] = recf
                        if rb_mode == "pe":
                            st["recr"] = expp.tile([1, NBLK], bf16,
                                                   tag="recr", name="recr")
                            nc.vector.tensor_copy(st["recr"][:], recf[:])

                    teng = nc.vector if tail_eng == "vector" else nc.gpsimd

                    def t_rb():
                        st["rbf"] = expp.tile([128, NBLK], f32, tag="rbf",
                                              name="rbf")
                        if rb_mode == "gpsimd":
                            nc.gpsimd.partition_broadcast(st["rbf"][:],
                                                          st["recf"][:])
                            return
                        rbps = ps_qk.tile([128, NBLK], f32, tag="qk",
                                          name="rb")
                        nc.tensor.matmul(rbps[:], onecol[:], st["recr"][:],
                                         start=True, stop=True)
                        teng.tensor_copy(st["rbf"][:], rbps[:])

                    def t_zr():
                        st["zr"] = [expp.tile([128, NBLK], bf16,
                                              tag=f"zr{zd}", name=f"zr{zd}")
                                    for zd in range(KD)]
                        for zd in range(KD):
                            nc.vector.tensor_mul(st["zr"][zd][:],
                                                 zps[zd][:], st["rbf"][:])

                    def t_out(do):
                        def f():
                            po = ps_qk.tile([128, NBLK], f32, tag="qk",
                                            name="po")
                            for zd in range(KD):
                                nc.tensor.matmul(
                                    po[:],
                                    wv[zd][:, do * 128:(do + 1) * 128],
                                    st["zr"][zd][:],
                                    start=(zd == 0), stop=(zd == KD - 1))
                            ot = expp.tile([128, NBLK], f32, tag="ot",
                                           name="ot")
                            teng.tensor_copy(ot[:], po[:])
                            nc.sync.dma_start(
                                outT_d[do * 128:(do + 1) * 128, nsl], ot[:])
                        return f

                    return [t_exr, t_sum, t_rb, t_zr,
                            t_out(0), t_out(1), t_out(2), t_out(3)]

                # ---- main: nb outer, mb inner ------------------------
                pending = []
                qT = phase_A(0)
                for nb in range(N_NB):
                    nsl = slice(nb * NBLK, (nb + 1) * NBLK)
                    if z_mode == "psum":
                        zps = [ps_z.tile([128, NBLK], f32, tag="z",
                                         name=f"z{zd}") for zd in range(KD)]
                    else:
                        zps = [expp.tile([128, NBLK], f32, tag=f"zacc{zd}",
                                         name=f"zacc{zd}")
                               for zd in range(KD)]
                    srp = (ps_sum.tile([2, NBLK], f32, tag="sm", name="sm")
                           if sum_eng == "pe" else None)
                    exaccw = (
                        [expp.tile([128, N_MC * NBLK], bf16, tag=f"exw{i}",
                                   name=f"exw{i}") for i in range(2)]
                        if sum_eng == "wide" else None)
                    exacc = (expp.tile([128, NBLK], f32, tag="exacc",
                                       name="exacc")
                             if sum_eng not in ("pe", "wide") else None)
                    exacc2 = (expp.tile([128, NBLK], f32, tag="exacc2",
                                        name="exacc2")
                              if sum_eng in ("split", "latesplit")
                              else None)
                    if not do_sums and exacc is not None:
                        nc.vector.memset(exacc[:], 1.0)
                        if exacc2 is not None:
                            nc.vector.memset(exacc2[:], 1.0)
                    if pending:
                        pending.pop(0)()     # t_exr of previous nb
                    ex_prev = None
                    for mb in range(N_MB):
                        if sum_eng == "wide":
                            exmb = expp.tile([128, N_MC * NBLK], bf16,
                                             tag="exmb", name="exmb",
                                             bufs=ex_bufs)
                            ex = [exmb[:, mc * NBLK:(mc + 1) * NBLK]
                                  for mc in range(N_MC)]
                        else:
                            ex = [expp.tile([128, NBLK], bf16,
                                            tag=f"ex{mc}",
                                            name=f"ex{mc}", bufs=ex_bufs)
                                  for mc in range(N_MC)]

                        def emit_add(kc_, t):
                            if sum_eng == "latesplit":
                                seng_ = (nc.vector if kc_ % 2 == 0
                                         else nc.gpsimd)
                            else:
                                seng_ = (nc.vector
                                         if sum_eng == "latevec"
                                         else nc.gpsimd)
                            acc = (exacc2 if (sum_eng == "latesplit" and
                                              kc_ % 2 == 1) else exacc)
                            lo = 1 if sum_eng == "latesplit" else 0
                            if kc_ <= lo:
                                seng_.tensor_copy(acc[:], t[:])
                            else:
                                seng_.tensor_add(acc[:], acc[:], t[:])
                        for mc in range(N_MC):
                            kc = mb * N_MC + mc
                            c0 = kc * 128
                            ps = ps_qk.tile([128, NBLK], f32, tag="qk",
                                            name="qk")
                            nc.tensor.matmul(ps[:], tau2[:, c0:c0 + 128],
                                             tau2[:, nsl],
                                             start=True, stop=not do_qk)
                            if do_qk:
                                for e in range(KD):
                                    nc.tensor.matmul(
                                        ps[:], hT[e][:, c0:c0 + 128],
                                        qT[e][:],
                                        start=False, stop=(e == KD - 1))
                            if do_exp:
                                nc.scalar.activation(ex[mc][:], ps[:], EXP,
                                                     bias=0.0, scale=SCALE)
                            if sum_eng == "split":
                                seng = nc.vector if kc % 2 == 0 else nc.gpsimd
                            elif sum_eng == "vector":
                                seng = nc.vector
                            else:
                                seng = nc.gpsimd
                            if (do_sums and do_exp and
                                    sum_eng in ("split", "vector",
                                                "gpsimd")):
                                acc = (exacc2 if (sum_eng == "split" and
                                                  kc % 2 == 1) else exacc)
                                if kc <= (1 if sum_eng == "split" else 0):
                                    seng.tensor_copy(acc[:], ex[mc][:])
                                else:
                                    seng.tensor_add(acc[:], acc[:],
                                                    ex[mc][:])
                            if (do_sums and do_exp and
                                    sum_eng.startswith("late") and
                                    ex_prev is not None):
                                emit_add((mb - 1) * N_MC + mc, ex_prev[mc])
                            if pending and (mb == 1 or
                                            (mb == 0 and mc < 3)):
                                pending.pop(0)()
                        if do_z and do_exp and z_mode == "psum":
                            for mc in range(N_MC):
                                for zd in range(KD):
                                    nc.tensor.matmul(
                                        zps[zd][:],
                                        hr[mb * N_MC + mc][:,
                                                           zd * 128:
                                                           (zd + 1) * 128],
                                        ex[mc][:],
                                        start=(mb == 0 and mc == 0),
                                        stop=(mb == N_MB - 1 and
                                              mc == N_MC - 1))
                                if sum_eng == "pe":
                                    nc.tensor.matmul(
                                        srp[:], ones[:], ex[mc][:],
                                        start=(mb == 0 and mc == 0),
                                        stop=(mb == N_MB - 1 and
                                              mc == N_MC - 1))
                        elif do_z and do_exp:
                            for zd in range(KD):
                                po = ps_z.tile([128, NBLK], f32, tag="z",
                                               name="po")
                                for mc in range(N_MC):
                                    nc.tensor.matmul(
                                        po[:],
                                        hr[mb * N_MC + mc][:,
                                                           zd * 128:
                                                           (zd + 1) * 128],
                                        ex[mc][:],
                                        start=(mc == 0),
                                        stop=(mc == N_MC - 1))
                                zeng = (nc.vector
                                        if (mb * KD + zd) % 2 == 0
                                        else nc.gpsimd)
                                if mb == 0:
                                    zeng.tensor_copy(zps[zd][:], po[:])
                                else:
                                    zeng.tensor_add(zps[zd][:], zps[zd][:],
                                                    po[:])
                        if do_sums and do_exp and sum_eng == "wide":
                            weng = nc.vector if mb % 2 == 0 else nc.gpsimd
                            wacc = exaccw[mb % 2]
                            if mb <= 1:
                                weng.tensor_copy(wacc[:], exmb[:])
                            else:
                                weng.tensor_add(wacc[:], wacc[:], exmb[:])
                        if mb == N_MB - 2 and nb < N_NB - 1:
                            qT_next = phase_A(nb + 1)
                        ex_prev = ex
                    if do_sums and do_exp and sum_eng.startswith("late"):
                        for mc in range(N_MC):
                            emit_add((N_MB - 1) * N_MC + mc, ex_prev[mc])
                    if do_tail and do_z and do_exp:
                        pending = make_tail(nb, zps, exacc, exacc2, srp,
                                            exaccw)
                    if nb == N_NB - 1:
                        for f in pending:
                            f()
                        pending = []
                    else:
                        qT = qT_next

            if isinstance(reps, str) and reps.startswith("unroll"):
                for _ in range(int(reps[6:])):
                    body()
            elif reps == 1:
                body()
            else:
                with tc.For_i(0, reps, 1):
                    body()

    nc.compile()
    _CACHE[key] = nc
    return nc


def _bf16(x: np.ndarray):
    import ml_dtypes
    return np.ascontiguousarray(x.astype(np.float32)).astype(
        ml_dtypes.bfloat16)


def _in_maps(h, Wq, Wk, Wv, tau):
    # G = Wq^T Wk host-side (weights-only preprocessing; q' = h G on device)
    g = _bf16(Wq.astype(np.float32).T @ Wk.astype(np.float32))   # [d, e]
    wvT = _bf16(np.ascontiguousarray(Wv.T))        # [d, e]
    taus = tau.astype(np.float32) / np.float32(np.sqrt(2.0))
    tau2 = _bf16(np.concatenate([taus.T, taus.T], axis=0))  # [2R, N]

    in_maps = []
    for c in range(NCORES):
        b, s = c // 2, c % 2
        hb = h[b].astype(np.float32)
        if s == 0:
            hperm = hb
            t2 = tau2
        else:
            hperm = np.concatenate([hb[NQ:], hb[:NQ]], axis=0)
            t2 = np.concatenate([tau2[:, NQ:], tau2[:, :NQ]], axis=1)
        in_maps.append({
            "hT": _bf16(np.ascontiguousarray(hperm.T)),
            "hrow": _bf16(hperm),
            "g": g, "wvT": wvT,
            "tau2": np.ascontiguousarray(t2),
        })
    return in_maps


def kernel(t, h, Wq, Wk, Wv, tau):
    from concourse.bass_utils import run_bass_kernel_spmd

    h = np.asarray(h, dtype=np.float32)
    Wq = np.asarray(Wq, dtype=np.float32)
    Wk = np.asarray(Wk, dtype=np.float32)
    Wv = np.asarray(Wv, dtype=np.float32)
    tau = np.asarray(tau, dtype=np.float32)

    nc = _build()
    in_maps = _in_maps(h, Wq, Wk, Wv, tau)
    try:
        res = run_bass_kernel_spmd(nc, in_maps, list(range(NCORES)))
    except Exception:
        # transient device/runtime hiccups usually clear on a retry
        res = run_bass_kernel_spmd(nc, in_maps, list(range(NCORES)))

    out = np.empty((B, N, D), dtype=np.float32)
    for c in range(NCORES):
        b, s = c // 2, c % 2
        out[b, s * NQ:(s + 1) * NQ, :] = res.results[c]["outT"].T
    return out
